# revision 19
# baseline (speedup 1.0000x reference)
"""Trainium2 Bass kernel for CausalSelfAttention (GQA + qk-rmsnorm + rope + head gating).

Sharding: 8 cores = 2 (batch) x 4 (kv-head groups). Each core computes the
full attention for one batch element and one kv-head group (4 q heads), plus
its slice of the output projection; partial projection outputs are summed on
the host.

Per-core on-device pipeline (all matmuls bf16 with fp32 PSUM accumulation):
  A) fused QKV+gate projection -> rmsnorm stats + rope (DVE/ACT) ->
     DMA-transpose q,k into head-dim-major layout
  B) flash-style causal attention per head in S^T layout:
     S^T = K @ Q^T, P = exp(S/sqrt(d)) (no max subtraction: |logits| <= 11.3),
     diagonal-block masking, Y = P @ [V | 1] (ones column gives the softmax
     denominator for free), per-token normalize * sigmoid gate,
     DMA-transpose y
  C) output projection partial: out = y @ Wproj_slice^T
"""

import numpy as np
import ml_dtypes
from contextlib import ExitStack

import concourse.bass as bass
import concourse.bacc as bacc
import concourse.mybir as mybir
import concourse.tile as tile
from concourse.bass_utils import run_bass_kernel_spmd

BF16 = mybir.dt.bfloat16
F32 = mybir.dt.float32
NPBF = ml_dtypes.bfloat16

B, T, D = 2, 2048, 2048
H, HKV, HD = 16, 4, 128
HALF = HD // 2
NHEAD = H // HKV          # q heads per core (group)
NT = T // 128             # 16 token tiles
NCHUNK = D // 128         # 16 contraction chunks
NQKV = NHEAD * HD + HD + HD + NHEAD   # 512 q + 128 k + 128 v + 4 gate = 772
ROPE_BASE = 10000.0
EPS = float(np.finfo(np.float32).eps)
SM_SCALE = 1.0 / float(np.sqrt(HD))

_CACHE = {}


def _build_program():
    nc = bacc.Bacc("TRN2", target_bir_lowering=False, debug=False,
                   enable_asserts=False, num_devices=8)

    xT_d = nc.dram_tensor("xT", [D, T], BF16, kind="ExternalInput").ap()
    wqkvg_d = nc.dram_tensor("wqkvg", [D, NQKV], BF16, kind="ExternalInput").ap()
    wproj_d = nc.dram_tensor("wproj", [NHEAD * HD, D], BF16, kind="ExternalInput").ap()
    cos_d = nc.dram_tensor("cosd", [T, HALF], F32, kind="ExternalInput").ap()
    sin_d = nc.dram_tensor("sind", [T, HALF], F32, kind="ExternalInput").ap()
    qgain_d = nc.dram_tensor("qgain", [1, NHEAD], F32, kind="ExternalInput").ap()
    gateb_d = nc.dram_tensor("gateb", [1, NHEAD], F32, kind="ExternalInput").ap()
    masks_d = nc.dram_tensor("masks", [128, 4, 512], BF16, kind="ExternalInput").ap()
    out_d = nc.dram_tensor("out", [T, D], F32, kind="ExternalOutput").ap()

    AF = mybir.ActivationFunctionType

    with tile.TileContext(nc) as tc, ExitStack() as ctx:
        consts = ctx.enter_context(tc.tile_pool(name="consts", bufs=1))

        # ---- resident tensors ----
        xT_sb = consts.tile([128, NCHUNK, T], BF16)
        for c in range(NCHUNK):
            nc.sync.dma_start(out=xT_sb[:, c, :], in_=xT_d[c * 128:(c + 1) * 128, :])
        wqkvg_sb = consts.tile([128, NCHUNK, NQKV], BF16)
        for c in range(NCHUNK):
            nc.sync.dma_start(out=wqkvg_sb[:, c, :],
                              in_=wqkvg_d[c * 128:(c + 1) * 128, :])
        wproj_sb = consts.tile([128, NHEAD, D], BF16)
        for h in range(NHEAD):
            nc.sync.dma_start(out=wproj_sb[:, h, :],
                              in_=wproj_d[h * 128:(h + 1) * 128, :])
        cos_sb = consts.tile([128, NT, HALF], F32)
        nc.sync.dma_start(out=cos_sb,
                          in_=cos_d.rearrange("(tt p) i -> p tt i", p=128))
        sin_sb = consts.tile([128, NT, HALF], F32)
        nc.sync.dma_start(out=sin_sb,
                          in_=sin_d.rearrange("(tt p) i -> p tt i", p=128))
        qgain_sb = consts.tile([128, NHEAD], F32)
        nc.sync.dma_start(out=qgain_sb, in_=bass.AP(
            tensor=qgain_d.tensor, offset=qgain_d.offset,
            ap=[[0, 128], [1, NHEAD]]))
        gateb_sb = consts.tile([128, NHEAD], F32)
        nc.sync.dma_start(out=gateb_sb, in_=bass.AP(
            tensor=gateb_d.tensor, offset=gateb_d.offset,
            ap=[[0, 128], [1, NHEAD]]))
        masks_sb = consts.tile([128, 4, 512], BF16)
        nc.sync.dma_start(out=masks_sb, in_=masks_d)

        qT_sb = consts.tile([128, NHEAD, T], BF16)   # head-dim-major q
        kT_sb = consts.tile([128, T], BF16)          # head-dim-major k
        v_sb = consts.tile([128, NT, HD + 1], BF16)  # [t | ones] per ki tile
        nc.vector.memset(v_sb[:, :, HD:HD + 1], 1.0)
        yT_sb = consts.tile([128, NHEAD, T], BF16)   # head-dim-major gated y
        gate_sb = consts.tile([128, NT, NHEAD], F32)
        eps_sb = consts.tile([128, 1], F32)
        nc.vector.memset(eps_sb, EPS)

        # =========== Phase A: QKV + gate, rms stats, rope, transpose ==========
        a_sb = ctx.enter_context(tc.tile_pool(name="phA", bufs=2))
        with tc.tile_pool(name="phA_ps", bufs=2, space="PSUM") as a_ps:
            for tt in range(NT):
                ts = slice(tt * 128, (tt + 1) * 128)
                qkv_a = a_ps.tile([128, 512], F32, tag="qkv_a")
                qkv_b = a_ps.tile([128, NQKV - 512], F32, tag="qkv_b")
                for c in range(NCHUNK):
                    lhs = xT_sb[:, c, ts]
                    nc.tensor.matmul(qkv_a, lhsT=lhs, rhs=wqkvg_sb[:, c, 0:512],
                                     start=(c == 0), stop=(c == NCHUNK - 1))
                    nc.tensor.matmul(qkv_b, lhsT=lhs, rhs=wqkvg_sb[:, c, 512:NQKV],
                                     start=(c == 0), stop=(c == NCHUNK - 1))

                # v tile (+ gate logits)
                nc.vector.tensor_copy(out=v_sb[:, tt, 0:HD], in_=qkv_b[:, 128:256])
                glog = a_sb.tile([128, NHEAD], F32, tag="glog")
                nc.vector.tensor_add(glog, qkv_b[:, 256:260], gateb_sb)
                nc.scalar.activation(out=gate_sb[:, tt, :], in_=glog, func=AF.Sigmoid)

                # rope on q (all 4 heads at once via broadcast cos/sin)
                qa3 = qkv_a.rearrange("p (h d) -> p h d", h=NHEAD)
                x1 = qa3[:, :, 0:HALF]
                x2 = qa3[:, :, HALF:HD]
                cos_t = cos_sb[:, tt, :]
                sin_t = sin_sb[:, tt, :]
                cos_b = bass.AP(tensor=cos_t.tensor, offset=cos_t.offset,
                                ap=[cos_t.ap[0], [0, NHEAD], cos_t.ap[1]])
                sin_b = bass.AP(tensor=sin_t.tensor, offset=sin_t.offset,
                                ap=[sin_t.ap[0], [0, NHEAD], sin_t.ap[1]])
                qrot = a_sb.tile([128, NHEAD, HD], F32, tag="qrot")
                u1 = a_sb.tile([128, NHEAD, HALF], F32, tag="u1")
                u2 = a_sb.tile([128, NHEAD, HALF], F32, tag="u2")
                nc.vector.tensor_mul(u1, x1, cos_b)
                nc.vector.tensor_mul(u2, x2, sin_b)
                nc.vector.tensor_add(qrot[:, :, 0:HALF], u1, u2)
                u3 = a_sb.tile([128, NHEAD, HALF], F32, tag="u3")
                u4 = a_sb.tile([128, NHEAD, HALF], F32, tag="u4")
                nc.vector.tensor_mul(u3, x2, cos_b)
                nc.vector.tensor_mul(u4, x1, sin_b)
                nc.vector.tensor_sub(qrot[:, :, HALF:HD], u3, u4)
                # rms scale + gain, cast to bf16
                # rope on k
                k1 = qkv_b[:, 0:HALF]
                k2 = qkv_b[:, HALF:HD]
                krot = a_sb.tile([128, HD], F32, tag="krot")
                w1 = a_sb.tile([128, HALF], F32, tag="w1")
                w2 = a_sb.tile([128, HALF], F32, tag="w2")
                nc.vector.tensor_mul(w1, k1, cos_t)
                nc.vector.tensor_mul(w2, k2, sin_t)
                nc.vector.tensor_add(krot[:, 0:HALF], w1, w2)
                nc.vector.tensor_mul(w1, k2, cos_t)
                nc.vector.tensor_mul(w2, k1, sin_t)
                nc.vector.tensor_sub(krot[:, HALF:HD], w1, w2)

                # mean-square per head from the (norm-preserving) rotated values
                msq = a_sb.tile([128, NHEAD + 1], F32, tag="msq")
                sqscr = a_sb.tile([128, NHEAD, HD], F32, tag="sqscr")
                sqscr_k = a_sb.tile([128, HD], F32, tag="sqscr_k")
                nc.vector.tensor_mul(sqscr, qrot, qrot)
                nc.vector.tensor_reduce(msq[:, 0:NHEAD], sqscr,
                                        axis=mybir.AxisListType.X,
                                        op=mybir.AluOpType.add)
                nc.vector.tensor_mul(sqscr_k, krot, krot)
                nc.vector.tensor_reduce(msq[:, NHEAD:NHEAD + 1], sqscr_k,
                                        axis=mybir.AxisListType.X,
                                        op=mybir.AluOpType.add)
                rtmp = a_sb.tile([128, NHEAD + 1], F32, tag="rtmp")
                nc.scalar.activation(out=rtmp, in_=msq, func=AF.Sqrt,
                                     scale=1.0 / HD, bias=eps_sb)
                r_all = a_sb.tile([128, NHEAD + 1], F32, tag="r_all")
                nc.vector.reciprocal(r_all, rtmp)
                rq = a_sb.tile([128, NHEAD], F32, tag="rq")
                nc.vector.tensor_mul(rq, r_all[:, 0:NHEAD], qgain_sb)
                k_stage = a_sb.tile([128, HD], BF16, tag="k_stage")
                nc.vector.tensor_scalar_mul(k_stage, krot, r_all[:, NHEAD:NHEAD + 1])

                # rms scale + gain, cast to bf16
                q_stage = a_sb.tile([128, NHEAD, HD], BF16, tag="q_stage")
                for h in range(NHEAD):
                    nc.vector.tensor_scalar_mul(q_stage[:, h, :], qrot[:, h, :],
                                                rq[:, h:h + 1])

                # transpose to head-dim-major via DMA xbar
                for h in range(NHEAD):
                    nc.sync.dma_start_transpose(out=qT_sb[:, h, ts],
                                                in_=q_stage[:, h, :])
                nc.sync.dma_start_transpose(out=kT_sb[:, ts], in_=k_stage)

        # =========== Phase B + C: attention, projection =======================
        b_sb = ctx.enter_context(tc.tile_pool(name="phB", bufs=3))
        c_sb = ctx.enter_context(tc.tile_pool(name="phC", bufs=3))
        with tc.tile_pool(name="phBC_ps", bufs=2, space="PSUM") as b_ps:
            for qc in range(4):
                qs_slice = slice(qc * 512, (qc + 1) * 512)
                nki = 4 * qc + 4
                for h in range(NHEAD):
                    y01 = b_ps.tile([128, 2, HD + 1], F32, tag="y01")
                    y23 = b_ps.tile([128, 2, HD + 1], F32, tag="y23")
                    for ki in range(nki):
                        m = ki - 4 * qc
                        nq = 512 - 128 * max(m, 0)
                        q_lo = qc * 512 + 128 * max(m, 0)
                        s_ps = b_ps.tile([128, 512], F32, tag="s")
                        nc.tensor.matmul(s_ps[:, 0:nq],
                                         lhsT=kT_sb[:, ki * 128:(ki + 1) * 128],
                                         rhs=qT_sb[:, h, q_lo:(qc + 1) * 512],
                                         start=True, stop=True)
                        p_sb = b_sb.tile([128, 512], BF16, tag="p")
                        nc.scalar.activation(out=p_sb[:, 0:nq], in_=s_ps[:, 0:nq],
                                             func=AF.Exp, scale=SM_SCALE)
                        if m >= 0:
                            nc.vector.tensor_mul(p_sb[:, 0:128], p_sb[:, 0:128],
                                                 masks_sb[:, 0, 0:128])
                        for qs in range(max(m, 0), 4):
                            ytile = y01 if qs < 2 else y23
                            pcol = (qs - max(m, 0)) * 128
                            nc.tensor.matmul(
                                ytile[:, qs % 2, :],
                                lhsT=p_sb[:, pcol:pcol + 128],
                                rhs=v_sb[:, ki, :],
                                start=(ki == 0 and qs % 2 == 0),
                                stop=(ki == 4 * qc + qs and qs % 2 == 1))
                    # normalize + gate + transpose (on the Scalar DMA queue,
                    # off the busy Sync queue)
                    for qs in range(4):
                        ytile = y01 if qs < 2 else y23
                        tt = qc * 4 + qs
                        rd = b_sb.tile([128, 1], F32, tag="rd")
                        nc.vector.reciprocal(rd, ytile[:, qs % 2, HD:HD + 1])
                        sc = b_sb.tile([128, 1], F32, tag="sc")
                        nc.vector.tensor_mul(sc, rd, gate_sb[:, tt, h:h + 1])
                        y_stage = b_sb.tile([128, HD], BF16, tag="y_stage")
                        nc.vector.tensor_scalar_mul(y_stage, ytile[:, qs % 2, 0:HD],
                                                    sc)
                        nc.scalar.dma_start_transpose(
                            out=yT_sb[:, h, tt * 128:(tt + 1) * 128], in_=y_stage)

                # Phase C for the token tiles finished by this qc
                for qs in range(4):
                    tt = qc * 4 + qs
                    ts = slice(tt * 128, (tt + 1) * 128)
                    for nch in range(4):
                        o_ps = b_ps.tile([128, 512], F32, tag="o")
                        for h in range(NHEAD):
                            nc.tensor.matmul(o_ps, lhsT=yT_sb[:, h, ts],
                                             rhs=wproj_sb[:, h,
                                                          nch * 512:(nch + 1) * 512],
                                             start=(h == 0), stop=(h == NHEAD - 1))
                        o_st = c_sb.tile([128, 512], F32, tag="o_st")
                        if nch % 2 == 0:
                            nc.scalar.copy(out=o_st, in_=o_ps)
                        else:
                            nc.vector.tensor_copy(out=o_st, in_=o_ps)
                        nc.sync.dma_start(out=out_d[ts, nch * 512:(nch + 1) * 512],
                                          in_=o_st)

    nc.compile()
    return nc


def _get_program():
    if "nc" not in _CACHE:
        _CACHE["nc"] = _build_program()
    return _CACHE["nc"]


def _host_prep(x, Wq, Wk, Wv, Wproj, q_gain, gate_w, gate_b):
    """Build the 8 per-core input maps."""
    f = np.float32
    x = np.asarray(x, f)
    WqT = np.asarray(Wq, f).T.astype(NPBF)       # [D, 2048]
    WkT = np.asarray(Wk, f).T.astype(NPBF)       # [D, 512]
    WvT = np.asarray(Wv, f).T.astype(NPBF)
    WpT = np.ascontiguousarray(np.asarray(Wproj, f).T.astype(NPBF))  # [D, D]
    gwT = np.asarray(gate_w, f).T.astype(NPBF)   # [D, 16]
    q_gain = np.asarray(q_gain, f)
    gate_b = np.asarray(gate_b, f)

    inv_freq = 1.0 / (ROPE_BASE ** (np.arange(0, HD, 2, dtype=f) / HD))
    tpos = np.arange(T, dtype=f)
    freqs = np.outer(tpos, inv_freq)
    cos = np.cos(freqs).astype(f)
    sin = np.sin(freqs).astype(f)

    kloc = np.arange(128)[:, None]
    qloc = np.arange(512)[None, :]
    masks = np.stack([(qloc >= kloc + 128 * m) for m in range(4)], axis=1)
    masks = masks.astype(NPBF)                   # [128, 4, 512]

    xT = [np.ascontiguousarray(x[b].T).astype(NPBF) for b in range(B)]

    in_maps = []
    for core in range(8):
        b, g = divmod(core, 4)
        wqkvg = np.concatenate([
            WqT[:, 512 * g:512 * (g + 1)],
            WkT[:, 128 * g:128 * (g + 1)],
            WvT[:, 128 * g:128 * (g + 1)],
            gwT[:, NHEAD * g:NHEAD * (g + 1)],
        ], axis=1)                               # [D, 772]
        in_maps.append({
            "xT": xT[b],
            "wqkvg": np.ascontiguousarray(wqkvg),
            "wproj": np.ascontiguousarray(WpT[512 * g:512 * (g + 1), :]),
            "cosd": cos,
            "sind": sin,
            "qgain": np.ascontiguousarray(q_gain[NHEAD * g:NHEAD * (g + 1)][None, :]),
            "gateb": np.ascontiguousarray(gate_b[NHEAD * g:NHEAD * (g + 1)][None, :]),
            "masks": masks,
        })
    return in_maps


def kernel(**inputs):
    nc = _get_program()
    in_maps = _host_prep(**inputs)
    res = run_bass_kernel_spmd(nc, in_maps, list(range(8)))
    parts = [r["out"] for r in res.results]
    out = np.empty((B, T, D), np.float32)
    for b in range(B):
        out[b] = parts[4 * b] + parts[4 * b + 1] + parts[4 * b + 2] + parts[4 * b + 3]
    return out
